# revision 32
# baseline (speedup 1.0000x reference)
"""Multi-head attention (B=4, N=2048, E=768, H=12, D=64) on 8 TRN2 NeuronCores.

Sharding: data-parallel on batch (4 batches x 2 cores each), tensor-parallel on
heads (6 heads per core).  Each core computes its heads' full NxN attention.
Partial output projections from the two cores of a batch are summed on the host.

Math simplifications (all exact):
  - softmax is shift invariant -> drop the +1.0 score bias and max-subtraction
    (scores are ~N(0,1); exp never overflows fp32)
  - K bias adds a per-query constant to every score row -> softmax invariant -> dropped
  - V bias shifts every attention output row by bv (softmax rows sum to 1)
    -> folded into the output bias on the host: b_eff = b_out + w_out @ bv
  - q scaling (1/8) folded into Wq and bq on the host

Device layout: scores are computed transposed (S^T = K Q^T, partition = key),
so P^T = exp(S^T) feeds the P@V matmul directly as the moving operand with
V as the stationary operand (O^T = (P V)^T accumulated over key blocks).
A ones-column appended to V (M=65) yields the softmax row-sums in the same
PE stream.  Matmul operands are bf16 (fp32 PSUM accumulation).

Schedule: the scalar engine's exp stream (192 x [128,1024], ~1.1us each) is
the steady-state bottleneck during attention; all projection work is either
in a DMA-overlapped pre-phase (QKV projections for pair 0 + the full V
projection) or interleaved into the attention jb loops as small units whose
PSUM tag matches the score buffer they borrow (placed right after that
buffer's exp so they sit in the tensor engine's natural stall slot).
Input DMAs are split across the two hardware DGE queues (sync + activation).
"""

import sys

if "/opt/trn_rl_repo" not in sys.path:
    sys.path.insert(0, "/opt/trn_rl_repo")

import numpy as np

B, N, E = 4, 2048, 768
H, D = 12, 64
HPC = 6                     # heads per core
FQK = HPC * D               # 384 q (or k) features per core
NCORES = 8
SCALE = D ** -0.5
PRECISION = "bf16"          # "bf16" | "f32r" (matmul operand dtype)

_CACHE = {}


def _build_bass():
    """Build the SPMD Bass program (same program on all 8 cores)."""
    if "nc" in _CACHE:
        return _CACHE["nc"]

    from contextlib import ExitStack

    import concourse.bass as bass
    import concourse.tile as tile
    from concourse import bacc, mybir

    f32 = mybir.dt.float32
    fmm = mybir.dt.bfloat16 if PRECISION == "bf16" else mybir.dt.float32r
    Exp = mybir.ActivationFunctionType.Exp

    nc = bacc.Bacc(
        "TRN2",
        target_bir_lowering=False,
        debug=False,
        num_devices=NCORES,
    )

    xT = nc.dram_tensor("xT", (E, N), fmm, kind="ExternalInput").ap()        # x[b].T
    wqkT = nc.dram_tensor("wqkT", (E, 2 * FQK), fmm, kind="ExternalInput").ap()
    bq = nc.dram_tensor("bq", (FQK, 1), f32, kind="ExternalInput").ap()
    wvT = nc.dram_tensor("wvT", (E, FQK), fmm, kind="ExternalInput").ap()
    woT = nc.dram_tensor("woT", (FQK, E), fmm, kind="ExternalInput").ap()
    # partial outputs ship as bf16 (halves the output DMA; the two cores'
    # partials are summed in fp32 on the host)
    yp = nc.dram_tensor("yp", (N, E), fmm, kind="ExternalOutput").ap()

    P = 128
    NCHUNK = 512            # token chunk for the projections
    IC = 1024               # query chunk in attention
    NPAIRS = HPC // 2       # head pairs (row-packed in the PE array)
    NB = N // P             # 16 key blocks

    with ExitStack() as ctx:
        tc = ctx.enter_context(tile.TileContext(nc))

        # ---- persistent tiles --------------------------------------------
        wpool = ctx.enter_context(tc.tile_pool(name="w", bufs=1))
        wqk_t = [wpool.tile([P, 2 * FQK], fmm, tag=f"wqk{t}", name=f"wqk{t}")
                 for t in range(6)]
        bqt = wpool.tile([P, 3], f32, tag="bq", name="bqt")
        xe_t = [wpool.tile([P, N], fmm, tag=f"xe{t}", name=f"xe{t}")
                for t in range(6)]
        wv_t = [wpool.tile([P, FQK], fmm, tag=f"wv{t}", name=f"wv{t}")
                for t in range(6)]
        wo_t = [wpool.tile([P, E], fmm, tag=f"wo{t}", name=f"wo{t}")
                for t in range(3)]
        scratch = wpool.tile([1, 2], f32, tag="scr", name="scratch")

        qk_pool = ctx.enter_context(tc.tile_pool(name="qk", bufs=1))
        # f-blocks 0..2 = q features (heads 2fb, 2fb+1), 3..5 = k features
        qkT_t = [
            qk_pool.tile([P, N], fmm, tag=f"qk{fb}", name=f"qkT{fb}")
            for fb in range(6)
        ]
        v_pool = ctx.enter_context(tc.tile_pool(name="v", bufs=1))
        # V' per key-block: [128 keys, 6*65] = per head 64 V cols + a ones col
        v_t = [
            v_pool.tile([P, HPC * 65], fmm, tag=f"v{nb}", name=f"vv{nb}")
            for nb in range(NB)
        ]

        psum = ctx.enter_context(tc.tile_pool(name="ps", bufs=1, space="PSUM"))
        TAGS4 = ("sA", "sB", "oA", "oB")

        pt_pool = ctx.enter_context(tc.tile_pool(name="pt", bufs=4))
        oT_pool = ctx.enter_context(tc.tile_pool(name="oT", bufs=1))
        # pair p partitions 0:64 = head 2p, 64:128 = head 2p+1
        oT_t = [
            oT_pool.tile([P, N], fmm, tag=f"oT{p}", name=f"oT{p}")
            for p in range(NPAIRS)
        ]
        nrm_pool = ctx.enter_context(tc.tile_pool(name="nrm", bufs=2))
        y_pool = ctx.enter_context(tc.tile_pool(name="y", bufs=3))

        # ---- ones columns for the row-sum trick (rest written by v-proj) --
        for nb in range(NB):
            v3 = v_t[nb].rearrange("p (h c) -> p h c", c=65)
            nc.gpsimd.memset(v3[:, :, 64:65], 1.0)

        # ---- input DMAs: split across the two hardware DGE queues --------
        # sync queue: weights;  activation queue: x^T (in token halves so the
        # first projection chains can start before the full x has landed)
        for t in range(6):
            nc.sync.dma_start(wqk_t[t][:], wqkT[t * P:(t + 1) * P, :])
        for fb in range(3):
            nc.sync.dma_start(bqt[:, fb:fb + 1], bq[fb * P:(fb + 1) * P, :])
        for t in range(6):
            nc.sync.dma_start(wv_t[t][:], wvT[t * P:(t + 1) * P, :])
        # x^T token-half 0 split across the activation HWDGE queue and the
        # gpsimd SWDGE queue (3rd ring) so the first projection chains can
        # start sooner; half 1 follows on the activation queue.
        for t in range(3):
            nc.scalar.dma_start(xe_t[t][:, 0:IC], xT[t * P:(t + 1) * P, 0:IC])
        for t in range(3, 6):
            nc.gpsimd.dma_start(xe_t[t][:, 0:IC], xT[t * P:(t + 1) * P, 0:IC])
        for t in range(6):
            nc.scalar.dma_start(
                xe_t[t][:, IC:N], xT[t * P:(t + 1) * P, IC:N])
        for t in range(3):
            nc.sync.dma_start(wo_t[t][:], woT[t * P:(t + 1) * P, :])

        # warm the Exp activation table while DMAs run
        nc.gpsimd.memset(scratch[:], 0.0)
        nc.scalar.activation(scratch[:, 0:1], scratch[:, 1:2], Exp)

        # ---- projection work units ---------------------------------------
        def qk_chain_units(fb, c4, tag):
            """QKV-projection chain for f-block fb, token chunk c4, as two
            3-matmul units sharing one PSUM tile (tag)."""
            st = {}
            n0 = c4 * NCHUNK

            def u1():
                st["ps"] = psum.tile([P, NCHUNK], f32, tag=tag, name="ps_qk")
                for et in range(3):
                    nc.tensor.matmul(
                        st["ps"][:],
                        lhsT=wqk_t[et][:, fb * P:(fb + 1) * P],
                        rhs=xe_t[et][:, n0:n0 + NCHUNK],
                        start=(et == 0),
                        stop=False,
                    )

            def u2():
                ps = st["ps"]
                for et in range(3, 6):
                    nc.tensor.matmul(
                        ps[:],
                        lhsT=wqk_t[et][:, fb * P:(fb + 1) * P],
                        rhs=xe_t[et][:, n0:n0 + NCHUNK],
                        start=False,
                        stop=(et == 5),
                    )
                dst = qkT_t[fb][:, n0:n0 + NCHUNK]
                if fb < 3:
                    nc.vector.tensor_scalar_add(dst, ps[:], bqt[:, fb:fb + 1])
                else:
                    nc.vector.tensor_copy(dst, ps[:])

            return [u1, u2]

        def v_chain_units(nb, tag):
            """V-projection for key block nb as two 3-matmul units."""
            st = {}

            def u1():
                st["ps"] = psum.tile([P, FQK], f32, tag=tag, name="ps_v")
                for et in range(3):
                    nc.tensor.matmul(
                        st["ps"][:],
                        lhsT=xe_t[et][:, nb * P:(nb + 1) * P],
                        rhs=wv_t[et][:],
                        start=(et == 0),
                        stop=False,
                    )

            def u2():
                ps = st["ps"]
                for et in range(3, 6):
                    nc.tensor.matmul(
                        ps[:],
                        lhsT=xe_t[et][:, nb * P:(nb + 1) * P],
                        rhs=wv_t[et][:],
                        start=False,
                        stop=(et == 5),
                    )
                v3 = v_t[nb].rearrange("p (h c) -> p h c", c=65)
                nc.vector.tensor_copy(
                    v3[:, :, 0:64],
                    ps.rearrange("p (h c) -> p h c", c=64),
                )

            return [u1, u2]

        yts = {}

        def op_unit(ic, nb2, half, tag):
            """Half of the output projection for one 128-token block."""
            n0 = ic * IC + nb2 * P
            f0 = half * 384

            def u():
                if half == 0:
                    yts[(ic, nb2)] = y_pool.tile([P, E], fmm, tag="y",
                                                 name="yt")
                yt = yts[(ic, nb2)]
                psy = psum.tile([P, 384], f32, tag=tag, name="psy")
                for dt3 in range(3):
                    nc.tensor.matmul(
                        psy[:],
                        lhsT=oT_t[dt3][:, n0:n0 + P],
                        rhs=wo_t[dt3][:, f0:f0 + 384],
                        start=(dt3 == 0),
                        stop=(dt3 == 2),
                    )
                nc.vector.tensor_copy(yt[:, f0:f0 + 384], psy[:])
                if ic == 1:
                    # tail: ship each half as soon as it lands, alternating
                    # the two DGE queues to shorten the final drain
                    eng = nc.scalar if (2 * nb2 + half) % 2 == 1 else nc.sync
                    eng.dma_start(yp[n0:n0 + P, f0:f0 + 384],
                                  yt[:, f0:f0 + 384])
                elif half == 1:
                    # during attention (ic=0) keep DMA issues off the
                    # activation queue — a waiting issue would stall exps
                    nc.sync.dma_start(yp[n0:n0 + P, :], yt[:])

            return u

        # ---- attention ----------------------------------------------------
        def attention(p, ic, fill=None, defer_norm=True, carry=(),
                      self_flush=False):
            """Attention for head pair p over query chunk ic.

            fill: dict jb -> list of (slot, unit) where slot "a"/"b" places
            the unit right after that score buffer's exp (the WAR on the
            borrowed PSUM tag then lines up with the tensor engine's natural
            wait-for-exp stall instead of delaying the next QK).

            defer_norm=True frees the oA/oB PSUM slots with plain copies and
            normalizes oT in place afterwards (off the inter-block critical
            path); the last block uses defer_norm=False so consumers of its
            oT aren't delayed by the extra copy."""
            fill = fill or {}
            i0 = ic * IC
            qT = qkT_t[p]
            kT = qkT_t[3 + p]
            oA = psum.tile([65, IC], f32, tag="oA", name="oA")
            oB = psum.tile([65, IC], f32, tag="oB", name="oB")
            for jb in range(NB):
                units = fill.get(jb, ())
                j0 = jb * P
                sA = psum.tile([P, IC], f32, tag="sA", name="sA")
                for u in range(IC // 512):
                    nc.tensor.matmul(
                        sA[:, u * 512:(u + 1) * 512],
                        lhsT=kT[0:64, j0:j0 + P],
                        rhs=qT[0:64, i0 + u * 512:i0 + (u + 1) * 512],
                        start=True,
                        stop=True,
                    )
                ptA = pt_pool.tile([P, IC], fmm, tag="ptA", name="ptA")
                nc.scalar.activation(ptA[:], sA[:], Exp)
                for slot, w in units:
                    if slot == "a":
                        w()
                sB = psum.tile([P, IC], f32, tag="sB", name="sB")
                for u in range(IC // 512):
                    nc.tensor.matmul(
                        sB[:, u * 512:(u + 1) * 512],
                        lhsT=kT[64:128, j0:j0 + P],
                        rhs=qT[64:128, i0 + u * 512:i0 + (u + 1) * 512],
                        start=True,
                        stop=True,
                    )
                ptB = pt_pool.tile([P, IC], fmm, tag="ptB", name="ptB")
                nc.scalar.activation(ptB[:], sB[:], Exp)
                for slot, w in units:
                    if slot == "b":
                        w()
                for u in range(IC // 512):
                    nc.tensor.matmul(
                        oA[:, u * 512:(u + 1) * 512],
                        lhsT=v_t[jb][:, (2 * p) * 65:(2 * p) * 65 + 65],
                        rhs=ptA[:, u * 512:(u + 1) * 512],
                        start=(jb == 0),
                        stop=(jb == NB - 1),
                    )
                    nc.tensor.matmul(
                        oB[:, u * 512:(u + 1) * 512],
                        lhsT=v_t[jb][:, (2 * p + 1) * 65:(2 * p + 1) * 65 + 65],
                        rhs=ptB[:, u * 512:(u + 1) * 512],
                        start=(jb == 0),
                        stop=(jb == NB - 1),
                    )
            # softmax normalization: O^T[d, i] /= rowsum[i].
            # NB: vector.reciprocal costs ~6.4ns/ELEMENT, so it must run on
            # the DMA-transposed [128, IC/128] layout, never on [1, IC].
            late = []
            for half, o_ps in ((0, oA), (1, oB)):
                dst = oT_t[p][half * 64:(half + 1) * 64, i0:i0 + IC]
                if defer_norm:
                    # free the oA/oB slots with plain copies first (the next
                    # block's first PV waits on them); scale in place later
                    nc.vector.tensor_copy(dst, o_ps[0:64, :])
                rs = nrm_pool.tile([1, IC], f32, tag="rs", name="rs")
                nc.vector.tensor_copy(rs[:], o_ps[64:65, :])
                rs128 = nrm_pool.tile([P, IC // P], f32, tag="rs128",
                                      name="rs128")
                nc.sync.dma_start(rs128[:], rs[:])
                rcp = nrm_pool.tile([P, IC // P], f32, tag="rcp", name="rcp")
                nc.vector.reciprocal(rcp[:], rs128[:])
                rcpf = nrm_pool.tile([1, IC], f32, tag="rcpf", name="rcpf")
                nc.sync.dma_start(rcpf[:], rcp[:])
                # full-height broadcast so the in-place mul's SBUF operands
                # share a start partition (walrus checkSBSameStartPartition)
                rb = nrm_pool.tile([P, IC], f32, tag="rb", name="rb")
                nc.gpsimd.partition_broadcast(rb[:], rcpf[:])
                if defer_norm:
                    # the in-place scale waits ~6us for the broadcast; on
                    # the in-order DVE queue that wait would block the next
                    # block's fill copies (whose score slots gate QK->exp).
                    # So its EMISSION is deferred: the caller emits it a
                    # block later, when the broadcast has long finished.
                    def mk(dst=dst, rb=rb, half=half):
                        def m():
                            nc.vector.tensor_mul(
                                dst, dst, rb[half * 64:(half + 1) * 64, :])
                        return m
                    late.append(mk())
                else:
                    nc.vector.tensor_mul(
                        dst, o_ps[0:64, :],
                        rb[half * 64:(half + 1) * 64, :])
            # previous block's deferred scales: inputs ready, no queue stall
            for m in carry:
                m()
            if self_flush:
                for m in late:
                    m()
                late = []
            return late

        # ---- schedule -----------------------------------------------------
        # Pre-phase (DMA-overlapped, scalar idle anyway): pair-0 q/k
        # projections + full V projection, 4-way PSUM tag rotation.
        tagc = [0]

        def nt():
            t = TAGS4[tagc[0] % 4]
            tagc[0] += 1
            return t

        def whole(units):
            def u():
                for f in units:
                    f()
            return u

        # Minimal pre-phase: just enough for att(0,0) jb 0..3 (kT chunk c0,
        # q chunks c0+c1, v blocks 0-1); everything else rides as fills.
        pre = []
        pre += qk_chain_units(3, 0, nt())
        pre += qk_chain_units(0, 0, nt())
        pre += qk_chain_units(0, 1, nt())
        pre += v_chain_units(0, nt())
        pre += v_chain_units(1, nt())
        for u in pre:
            u()

        def build_fill(assign):
            """assign: list of (jb, slot, unit_builder) with slot "a"/"b";
            the borrowed psum tag always matches the slot."""
            fl = {}
            for jb, slot, units in assign:
                fl.setdefault(jb, []).append((slot, whole(units)))
            return fl

        def qk_u(fb, c4, slot):
            return qk_chain_units(fb, c4, "sA" if slot == "a" else "sB")

        def v_u(nb, slot):
            return v_chain_units(nb, "sA" if slot == "a" else "sB")

        # att(0,0): remaining kT/q chunks early, V blocks >=1 jb ahead of use
        f00 = build_fill([
            (0, "a", qk_u(3, 1, "a")), (0, "b", v_u(2, "b")),
            (1, "a", v_u(3, "a")), (1, "b", v_u(4, "b")),
            (2, "a", qk_u(3, 2, "a")), (2, "b", v_u(5, "b")),
            (3, "a", v_u(6, "a")), (3, "b", v_u(7, "b")),
            (4, "a", qk_u(3, 3, "a")), (4, "b", v_u(8, "b")),
            (5, "b", v_u(9, "b")),
            (6, "a", v_u(10, "a")),
            (7, "b", v_u(11, "b")),
            (8, "a", v_u(12, "a")),
            (9, "b", v_u(13, "b")),
            (10, "a", qk_u(0, 2, "a")), (10, "b", v_u(14, "b")),
            (11, "b", v_u(15, "b")),
            (12, "a", qk_u(0, 3, "a")),
        ])
        # att(0,1): pair-1 projections that must exist before att(1,0)
        f01 = build_fill([
            (1, "a", qk_u(4, 0, "a")),
            (5, "b", qk_u(4, 1, "b")),
            (9, "a", qk_u(1, 0, "a")),
            (13, "b", qk_u(1, 1, "b")),
        ])
        # att(1,0): late chunks of pair-1 + first pair-2 k chunk
        f10 = build_fill([
            (0, "a", qk_u(4, 2, "a")),
            (2, "b", qk_u(4, 3, "b")),
            (6, "a", qk_u(1, 2, "a")),
            (10, "b", qk_u(1, 3, "b")),
            (12, "a", qk_u(5, 0, "a")),
        ])
        # att(1,1): rest of pair-2 projections (needed before att(2,0))
        f11 = build_fill([
            (0, "a", qk_u(5, 1, "a")),
            (2, "b", qk_u(5, 2, "b")),
            (4, "a", qk_u(5, 3, "a")),
            (6, "b", qk_u(2, 0, "b")),
            (8, "a", qk_u(2, 1, "a")),
            (10, "b", qk_u(2, 2, "b")),
            (12, "a", qk_u(2, 3, "a")),
        ])

        def op_fill(ic):
            """out_proj(ic) as fills.  Starts at jb=3: the oT tiles it reads
            come from the previous block's deferred normalization, which
            needs a few microseconds after that block's last PV."""
            fl = {}
            for i in range(2 * (IC // P)):
                nb2, half = i // 2, i % 2
                j = 3 + ((i * 13) // 16)
                tag = "sA" if i % 2 == 0 else "sB"
                fl.setdefault(j, []).append(
                    ("a" if i % 2 == 0 else "b",
                     op_unit(ic, nb2, half, tag)))
            return fl

        lt = attention(0, 0, fill=f00)
        lt = attention(0, 1, fill=f01, carry=lt)
        lt = attention(1, 0, fill=f10, carry=lt)
        lt = attention(1, 1, fill=f11, carry=lt)
        lt = attention(2, 0, carry=lt, self_flush=True)
        attention(2, 1, fill=op_fill(0), carry=lt, self_flush=True)

        # ---- tail: out_proj for the last query chunk -----------------------
        # Each unit is split: p1 = the two matmuls reading pair-0/1 oT
        # (available long before), p2 = the pair-2 matmul + copy + DMA.
        # The p1s execute during the last block's normalization chain,
        # keeping the tensor engine warm (a cold PE runs matmuls ~3x slower).
        def op_split(nb2, half, tag):
            n0 = IC + nb2 * P
            f0 = half * 384
            st = {}

            def p1():
                if half == 0:
                    yts[(1, nb2)] = y_pool.tile([P, E], fmm, tag="y",
                                                name="yt")
                st["ps"] = psum.tile([P, 384], f32, tag=tag, name="psy")
                for dt3 in range(2):
                    nc.tensor.matmul(
                        st["ps"][:],
                        lhsT=oT_t[dt3][:, n0:n0 + P],
                        rhs=wo_t[dt3][:, f0:f0 + 384],
                        start=(dt3 == 0),
                        stop=False,
                    )

            def p2():
                yt = yts[(1, nb2)]
                nc.tensor.matmul(
                    st["ps"][:],
                    lhsT=oT_t[2][:, n0:n0 + P],
                    rhs=wo_t[2][:, f0:f0 + 384],
                    start=False,
                    stop=True,
                )
                nc.vector.tensor_copy(yt[:, f0:f0 + 384], st["ps"][:])
                eng = nc.scalar if (2 * nb2 + half) % 2 == 1 else nc.sync
                eng.dma_start(yp[n0:n0 + P, f0:f0 + 384], yt[:, f0:f0 + 384])

            return p1, p2

        tunits = [op_split(nb2, half, TAGS4[(2 * nb2 + half) % 4])
                  for nb2 in range(IC // P) for half in range(2)]
        for i in range(4):
            tunits[i][0]()
        for i in range(len(tunits)):
            tunits[i][1]()
            if i + 4 < len(tunits):
                tunits[i + 4][0]()

    nc.compile()
    _CACHE["nc"] = nc
    return nc


def _shard_inputs(x_q, w_qkv, b_qkv, w_out):
    """Build the 8 per-core input maps (numpy, host side)."""
    if PRECISION == "bf16":
        import ml_dtypes

        mm_np = ml_dtypes.bfloat16
    else:
        mm_np = np.float32

    def cmm(a):
        return np.ascontiguousarray(a.astype(mm_np))

    in_maps = []
    for c in range(NCORES):
        b = c // 2
        h0 = (c % 2) * HPC
        qs = slice(h0 * D, h0 * D + FQK)
        ks = slice(E + h0 * D, E + h0 * D + FQK)
        vs = slice(2 * E + h0 * D, 2 * E + h0 * D + FQK)
        wq = w_qkv[qs] * SCALE                       # (384, 768)
        wk = w_qkv[ks]
        wv = w_qkv[vs]
        in_maps.append({
            "xT": cmm(x_q[b].T),                                     # (768, 2048)
            "wqkT": cmm(np.concatenate([wq, wk], axis=0).T),         # (768, 768)
            "bq": np.ascontiguousarray(
                (b_qkv[qs] * SCALE).reshape(FQK, 1)),                # (384, 1)
            "wvT": cmm(wv.T),                                        # (768, 384)
            "woT": cmm(w_out[:, h0 * D:h0 * D + FQK].T),
        })
    return in_maps


def kernel(x_q, w_qkv, b_qkv, w_out, b_out, _trace=False, _tmpdir=None):
    x_q = np.asarray(x_q, dtype=np.float32)
    w_qkv = np.asarray(w_qkv, dtype=np.float32)
    b_qkv = np.asarray(b_qkv, dtype=np.float32)
    w_out = np.asarray(w_out, dtype=np.float32)
    b_out = np.asarray(b_out, dtype=np.float32)

    from concourse.bass_utils import run_bass_kernel_spmd

    nc = _build_bass()
    in_maps = _shard_inputs(x_q, w_qkv, b_qkv, w_out)
    res = run_bass_kernel_spmd(
        nc, in_maps, core_ids=list(range(NCORES)), trace=_trace, tmpdir=_tmpdir
    )
    _CACHE["last_result"] = res

    # host unshard: sum the two head-shards of each batch, add the folded bias
    bv = b_qkv[2 * E:]                       # v bias, folded through w_out
    b_eff = b_out + w_out @ bv               # (768,)
    y = np.empty((B, N, E), dtype=np.float32)
    for b in range(B):
        y[b] = (
            res.results[2 * b]["yp"].astype(np.float32)
            + res.results[2 * b + 1]["yp"].astype(np.float32)
            + b_eff
        )
    return y
